# revision 31
# baseline (speedup 1.0000x reference)
"""Conv2D 3x3 (stride 1, pad 1) Trainium2 Bass kernel — 1D Winograd F(2,3).

Problem: x (16,128,56,56) f32 * W (256,128,3,3) + b (256,) -> (16,256,56,56) f32

Strategy:
  - Data parallel over batch: 8 cores x 2 images; W/b replicated.
  - 1D Winograd F(2,3) along the W (column) axis: host transforms x into 4
    planes xt_p[ci, 58, 28] (bf16) and W into U[kh,p][ci,co] (bf16). Device
    computes m_p[r,j] = sum_kh U(kh,p)^T @ xt_p[r+kh, :] as 3-matmul PSUM
    accumulations (N=392 = 14 rows x 28 cols per group), then combines
      Y_even = m0+m1+m2+b ,  Y_odd = m1-m2-m3+b
    across Scalar (s=act(m1+b), c2=act(m2)), Vector (p=m0+s, q=s-c2, Yo=q-m3;
    vector owns every PSUM-bank-releasing op) and GpSimd (Ye=p+c2, SBUF-only)
    — all drain engines stay under the PE's ~32.3us of matmul work (vs 47.6us
    PE floor for direct conv).
  - PE work per core: 192 matmuls x N=392 = 75k cycles (2/3 of direct conv);
    measured stream is ~97% back-to-back (168ns/matmul vs 163 ideal).
  - Startup: the 8-core HBM burst gates the first real matmul (~6us); dummy
    warmup matmuls cover that window and hold the PE pstate up. Only the
    data needed before ~10us is in the burst; later planes (img0 B-pieces,
    U chunk1, img1) trickle in behind the first output DMAs.
  - Output written bf16 (tolerance allows; halves out-DMA); host interleaves
    even/odd columns and upcasts to f32.
  - Measured: ~51.5us mean / ~52.5us max-core (baseline direct conv: 66.5us);
    ~10.5us of that is fixed framework start/teardown, 12.4us for a null
    kernel.
"""

import os
import sys

for _p in ("/opt/trn_rl_repo", os.path.expanduser("~/.axon_site/_ro/trn_rl_repo")):
    if os.path.isdir(_p) and _p not in sys.path:
        sys.path.insert(0, _p)
        break

import numpy as np
import ml_dtypes

B, C_IN, H, W_SP = 16, 128, 56, 56
C_OUT, KH, KW = 256, 3, 3
N_CORES = 8
B_PER_CORE = B // N_CORES          # 2
CHUNKS = C_OUT // 128              # 2
HP = H + 2                         # 58 padded rows
NJ = 28                            # output column pairs
XT_COLS = HP * NJ                  # 1624 per plane
ROWS_PER_G = 14                    # output rows per group
NG = H // ROWS_PER_G               # 4 groups
NT = ROWS_PER_G * NJ               # 392 = matmul moving N
U_PLANE = 128                      # cols per U plane (co)
U_CHUNK = 4 * KH * U_PLANE         # 1536 cols per chunk: p-major, kh, co
N_WARM = 42                        # pstate-ramp dummy matmuls
P_ORDER = (1, 2, 0, 3)             # m-plane fill order (m1,m2 first for drain)

_CACHE = {}


def _build(null=False):
    from concourse import bacc, mybir
    import concourse.tile as tile

    f32 = mybir.dt.float32
    bf16 = mybir.dt.bfloat16
    Ident = mybir.ActivationFunctionType.Identity
    ADD = mybir.AluOpType.add
    SUB = mybir.AluOpType.subtract

    nc = bacc.Bacc(trn_type="TRN2", name="conv_wino")
    xt_h = nc.dram_tensor("xt", [B_PER_CORE, 4, C_IN, XT_COLS], bf16,
                          kind="ExternalInput")
    w_h = nc.dram_tensor("wt", [C_IN, CHUNKS * U_CHUNK], bf16,
                         kind="ExternalInput")
    b_h = nc.dram_tensor("bias", [128, CHUNKS], f32, kind="ExternalInput")
    # out[img][chunk][co][eo][group][392] bf16
    o_h = nc.dram_tensor("out", [B_PER_CORE, CHUNKS, 128, 2, NG, NT], bf16,
                         kind="ExternalOutput")

    with tile.TileContext(nc) as tc:
        with tc.tile_pool(name="const", bufs=1) as cpool, \
             tc.tile_pool(name="dr", bufs=4) as dpool, \
             tc.tile_pool(name="ps", bufs=8, space="PSUM") as ppool:
            b_sb = cpool.tile([128, CHUNKS], f32)

            if null:
                nc.sync.dma_start(out=b_sb[:, :], in_=b_h[:, :])
                zt = cpool.tile([128, NT], bf16)
                nc.vector.memset(zt[:, :], 0)
                nc.sync.dma_start(out=o_h[0, 0, :, 0, 0, :], in_=zt[:, :])
                nc.finalize()
                return nc

            # PE warmup: dummy matmuls runnable immediately (no DMA dep);
            # ramp the PE pstate while the first xt/U DMAs land.
            warm = cpool.tile([128, 128], bf16)
            nc.vector.memset(warm[:, :], 0)
            wps = ppool.tile([128, NT], f32, name="m")
            for _ in range(N_WARM):
                nc.tensor.matmul(wps[:, :128], warm[:, :], warm[:, :],
                                 start=True, stop=True)

            w_sb = cpool.tile([C_IN, CHUNKS * U_CHUNK], bf16)
            xts = [[cpool.tile([C_IN, XT_COLS], bf16, name=f"xt{i}{p}")
                    for p in range(4)] for i in range(B_PER_CORE)]

            # Critical path: first matmul needs U(chunk0, p=1) + xt[0][1].
            # sync queue: img0 xt planes split in row-pieces (first pieces
            # unblock groups 0-1 early); scalar queue: U pieces + bias, then
            # img1 xt planes.
            SPLIT = 30 * NJ  # rows 0-29 cover groups 0-1
            nc.scalar.dma_start(out=w_sb[:, 384:512],
                                in_=w_h[:, 384:512])            # c0 p1 kh0
            nc.scalar.dma_start(out=w_sb[:, 512:768],
                                in_=w_h[:, 512:768])            # c0 p1 kh1,2
            nc.scalar.dma_start(out=w_sb[:, 768:1536],
                                in_=w_h[:, 768:1536])           # chunk0 p2,p3
            nc.scalar.dma_start(out=w_sb[:, :384], in_=w_h[:, :384])  # c0 p0
            nc.scalar.dma_start(out=b_sb[:, :], in_=b_h[:, :])
            FIRST = 16 * NJ  # rows 0-15: all of group 0's p1 reads
            nc.sync.dma_start(out=xts[0][1][:, :FIRST], in_=xt_h[0, 1, :, :FIRST])
            for p in P_ORDER:
                if p == 1:
                    nc.sync.dma_start(out=xts[0][1][:, FIRST:SPLIT],
                                      in_=xt_h[0, 1, :, FIRST:SPLIT])
                else:
                    nc.sync.dma_start(out=xts[0][p][:, :SPLIT],
                                      in_=xt_h[0, p, :, :SPLIT])
            nc.sync.dma_start(out=xts[0][1][:, SPLIT:], in_=xt_h[0, 1, :, SPLIT:])

            for img in range(B_PER_CORE):
                for chunk in range(CHUNKS):
                    for g in range(NG):
                        ps = {}
                        for p in P_ORDER:
                            ps[p] = ppool.tile([128, NT], f32, name="m")
                            for kh in range(KH):
                                wcol = chunk * U_CHUNK + p * (KH * 128) + kh * 128
                                r0 = (ROWS_PER_G * g + kh) * NJ
                                nc.tensor.matmul(
                                    ps[p][:, :],
                                    w_sb[:, wcol:wcol + 128],
                                    xts[img][p][:, r0:r0 + NT],
                                    start=(kh == 0),
                                    stop=(kh == KH - 1),
                                )
                        s = dpool.tile([128, NT], f32, name="s")
                        c2 = dpool.tile([128, NT], f32, name="c2")
                        pt = dpool.tile([128, NT], f32, name="pt")
                        q = dpool.tile([128, NT], f32, name="q")
                        yeo = dpool.tile([128, 2 * NT], bf16, name="yeo")
                        ye = yeo[:, :NT]
                        yo = yeo[:, NT:]
                        # scalar: PSUM reads of m1 (with bias) and m2
                        nc.scalar.activation(s[:, :], ps[1][:, :], Ident,
                                             bias=b_sb[:, chunk:chunk + 1])
                        nc.scalar.activation(c2[:, :], ps[2][:, :], Ident)
                        # vector owns every PSUM-bank-releasing op (fast, so
                        # banks free quickly for the PE): p = m0+s, q = s-c2,
                        # Yo = q-m3.  gpsimd does Ye = p+c2 (SBUF-only, off
                        # the bank-release critical path).
                        nc.vector.tensor_tensor(pt[:, :], ps[0][:, :], s[:, :], ADD)
                        nc.vector.tensor_tensor(q[:, :], s[:, :], c2[:, :], SUB)
                        tail2 = (img == B_PER_CORE - 1 and chunk == CHUNKS - 1
                                 and g >= NG - 2)
                        yeq = nc.vector if tail2 else nc.gpsimd
                        yeq.tensor_tensor(ye, pt[:, :], c2[:, :], ADD)
                        nc.vector.tensor_tensor(yo, q[:, :], ps[3][:, :], SUB)
                        nc.sync.dma_start(out=o_h[img, chunk, :, :, g, :],
                                          in_=yeo[:, :])
                        # deferred input DMAs, in need order: img0 B-pieces
                        # (needed ~7us), U chunk1 (~13us), img1 planes (~28us)
                        if img == 0:
                            slot = chunk * NG + g
                            if slot == 0:
                                for pb in (2, 0, 3):
                                    nc.sync.dma_start(
                                        out=xts[0][pb][:, SPLIT:],
                                        in_=xt_h[0, pb, :, SPLIT:])
                            elif slot == 1:
                                nc.sync.dma_start(out=w_sb[:, 1536:],
                                                  in_=w_h[:, 1536:])
                            elif slot <= 5:
                                p2 = P_ORDER[slot - 2]
                                nc.sync.dma_start(out=xts[1][p2][:, :],
                                                  in_=xt_h[1, p2, :, :])
    nc.finalize()
    return nc


def _get_nc(null=False):
    key = ("nc", null)
    if key not in _CACHE:
        _CACHE[key] = _build(null=null)
    return _CACHE[key]


def kernel(x, W, b, _trace=False):
    from concourse.bass_utils import run_bass_kernel_spmd

    x = np.asarray(x, dtype=np.float32)
    W = np.asarray(W, dtype=np.float32)
    b = np.asarray(b, dtype=np.float32)
    bf = ml_dtypes.bfloat16

    # --- host input transform (f32 math, store bf16) ---
    xp = np.zeros((B, C_IN, HP, HP), np.float32)
    xp[:, :, 1:1 + H, 1:1 + W_SP] = x
    c0 = xp[:, :, :, 0:56:2]
    c1 = xp[:, :, :, 1:57:2]
    c2 = xp[:, :, :, 2:58:2]
    c3 = np.zeros_like(c0)
    c3[:, :, :, :27] = xp[:, :, :, 3:57:2]
    # planes p=0..3: [B, CI, 4, 58, 28]
    xt = np.stack([c0 - c2, c1 + c2, c2 - c1, c1 - c3], axis=2).astype(bf)
    xt = np.ascontiguousarray(xt.reshape(B, C_IN, 4, XT_COLS).transpose(0, 2, 1, 3))
    xt = xt.reshape(B, 4, C_IN, XT_COLS)

    # --- host weight transform: U[p,kh][ci,co], layout [ci, chunk,p,kh,co] ---
    G = np.array([[1, 0, 0], [0.5, 0.5, 0.5], [0.5, -0.5, 0.5], [0, 0, 1]],
                 np.float32)
    U = np.einsum("pk,oihk->ihpo", G, W)        # [ci, kh, p, co]
    wt = (U.transpose(0, 2, 1, 3)               # [ci, p, kh, co]
          .reshape(C_IN, 4, KH, CHUNKS, 128)
          .transpose(0, 3, 1, 2, 4)             # [ci, chunk, p, kh, co]
          .reshape(C_IN, CHUNKS * U_CHUNK).astype(bf))
    wt = np.ascontiguousarray(wt)
    bias = np.ascontiguousarray(b.reshape(CHUNKS, 128).T)

    nc = _get_nc()
    in_maps = [
        {"xt": xt[c * B_PER_CORE:(c + 1) * B_PER_CORE], "wt": wt, "bias": bias}
        for c in range(N_CORES)
    ]
    res = run_bass_kernel_spmd(nc, in_maps, core_ids=list(range(N_CORES)),
                               trace=_trace)
    # gather: res out [2, 2, 128, 2, 4, 392] bf16 per core
    full = np.empty((B, C_OUT, H, W_SP), np.float32)
    for c in range(N_CORES):
        o = np.asarray(res.results[c]["out"]).astype(np.float32)
        o = o.reshape(B_PER_CORE, CHUNKS, 128, 2, NG, ROWS_PER_G, NJ)
        # -> [img, chunk, co, group, rows, j, eo]
        o = o.transpose(0, 1, 2, 4, 5, 6, 3)
        full[c * B_PER_CORE:(c + 1) * B_PER_CORE] = o.reshape(
            B_PER_CORE, C_OUT, H, W_SP)
    if _trace:
        _CACHE["last_results"] = res
    return full
